# revision 54
# baseline (speedup 1.0000x reference)
"""Trainium2 Bass kernel for nn_DotProductAttention_6030134084023.

reference: softmax(mask(Q @ K^T / sqrt(64), valid_lens)) @ V
  query/key/value: [64, 1024, 64] f32, valid_lens: [64] int32 -> [64, 1024, 64] f32

Strategy
--------
Batch dim sharded across the 8 NeuronCores. The host sorts batches by
valid_len (descending) and deals them round-robin, so slot s on every core
holds similar-length batches; the kernel is compiled per call with a
per-slot chunk count (chunks past a slot's max length have an all-zero
mask so skipping them is exactly lossless).

Per-core dataflow per batch (k on SBUF partitions for scores; q on
partitions for the PV result, so softmax normalization is per-partition):

  ST[k, q]   = KT_chunk.T @ QT        PE f32r; two K=64 k-chunks run
                                      CONCURRENTLY as row-packed tiles
  EST        = exp(0.125 * ST)        ScalarE exp -> bf16
  UTT[q, d'] += EST_blk.T @ Vm_chunk  PE bf16; est [128,128] stationary,
                                      Vm [128, 65] moving, accumulated
                                      over k-chunks in PSUM

with Vm_chunk = [V_chunk * mask | mask] (bf16 [128, 65]): the valid_lens
mask is applied to the small V' operand, so EST needs no masking and UTT
column 64 accumulates the masked softmax denominator PER PARTITION (q).
Postprocess per (slot, q-half) is then tiny: a strided [128,4] den
gather + reciprocal_approx_fast on DVE, ONE tensor_tensor multiply with
the reciprocal broadcast along the free dim, and one straight DMA per
slot to a blocked [128, 8*65] DRAM layout the host un-blocks (layout
work only on host). All input DMA descriptors are generated up front on
the sync queue into per-slot resident tiles (no pool-rotation waits);
UTT emission trails its group by one so the PE's in-order queue always
holds an independent ST ahead of each est dependency; postprocess
stages pop two-per-group behind the steady pipeline.

A Schraudolph bit-trick DVE exp (int16 affine -> bitcast bf16) can
offload the last pair of long halves from ScalarE (DVE_EXP) — measured
net-negative on the pipeline, so it ships disabled.
"""

import numpy as np

import concourse.bass as bass
import concourse.bacc as bacc
import concourse.tile as tile
from concourse import mybir
from concourse import bass_utils

F32 = mybir.dt.float32
F32R = mybir.dt.float32r
BF16 = mybir.dt.bfloat16
I16 = mybir.dt.int16
I32 = mybir.dt.int32
AF = mybir.ActivationFunctionType
ALU = mybir.AluOpType

NCORES = 8
B = 64
S = 1024
D = 64
BPC = B // NCORES  # 8 batch slots per core
KC = S // 128  # 8 k-chunks of 128
QH = 512  # q-half (max fp32 matmul moving dim)

# Schraudolph bf16 exp: exp(0.125*s) ~= bitcast_bf16(int16(SCH_A*s + SCH_B))
SCH_A = 128.0 / float(np.log(2)) * 0.125
SCH_B = 16250.4
# offload exp of the last pair of each (slot, q-half) to the DVE
# Schraudolph approximation (only when the half has >= MIN_PAIRS_FOR_DVE
# pairs, bounding the approximated softmax-mass fraction per batch)
DVE_EXP = False
MIN_PAIRS_FOR_DVE = 3

_BUILD_CACHE = {}
_RUN_KWARGS = {}  # test harness may set {"trace": True}
_LAST_RES = None


def _build(nprocs, nreals):
    """nprocs[s]: even number of 128-chunks to process for batch slot s.
    nreals[s]: chunks with any valid key across the slot's cores (UT
    matmuls and mask work for chunks >= nreals[s] are skipped)."""
    nc = bacc.Bacc("TRN2", target_bir_lowering=False, debug=False, num_devices=NCORES)
    qt = nc.dram_tensor("qt", [BPC, D, S], F32, kind="ExternalInput").ap()
    kt = nc.dram_tensor("kt", [BPC, D, S], F32, kind="ExternalInput").ap()
    v = nc.dram_tensor("v", [BPC, S, D], F32, kind="ExternalInput").ap()
    vl = nc.dram_tensor("vl", [1, BPC], I32, kind="ExternalInput").ap()
    # blocked output layout: ot[b, p, jj*65+c] = O[b, jj*128+p, c] for c<64
    # (contiguous per partition -> cheap DMA; host un-blocks)
    ot = nc.dram_tensor("ot", [BPC, 128, KC * (D + 1)], F32, kind="ExternalOutput").ap()

    # shortest slots first: their small input DMAs land quickly so compute
    # starts early, while the descriptor-heavy V loads of the long slots
    # prefetch under earlier slots' compute
    slot_order = [6, 7, 5, 4, 3, 2, 1, 0]

    with tile.TileContext(nc) as tc:
        with (
            tc.tile_pool(name="const", bufs=1) as constp,
            tc.tile_pool(name="resid", bufs=1) as residp,
            tc.tile_pool(name="estp", bufs=6) as estp,
            tc.tile_pool(name="post", bufs=3) as postp,
            tc.tile_pool(name="stp", bufs=2, space="PSUM") as stp,
            tc.tile_pool(name="utp", bufs=4, space="PSUM") as utp,
        ):
            # ---- kick off the exp table load immediately (runs ~2.7us
            # concurrently with the initial DMAs); exp(0*x)=1, no Inf risk
            warm = constp.tile([1, 8], F32)
            nc.scalar.activation(out=warm[:], in_=warm[:], func=AF.Exp, scale=0.0)

            # ---- ALL input DMAs up front ------------------------------
            # Every slot's tiles are resident (no pool rotation), so the
            # sync queue streams descriptor generation for the whole kernel
            # with zero waits, in slot_order, overlapped with compute. The
            # descriptor-heavy V loads of big slots sit at the end.
            slot_res = {}

            def emit_inputs(b):
                nreal = nreals[b]
                kw = nreal * 128
                qt2 = residp.tile([128, S], F32R, tag=f"qt{b}", name=f"qt2_{b}")
                kt2 = residp.tile([128, S], F32R, tag=f"kt{b}", name=f"kt2_{b}")
                for half in (slice(0, 64), slice(64, 128)):
                    nc.sync.dma_start(out=qt2[half, :], in_=qt[b].bitcast(F32R))
                    nc.sync.dma_start(
                        out=kt2[half, 0:kw], in_=kt[b, :, 0:kw].bitcast(F32R)
                    )
                slot_res[b] = dict(qt2=qt2, kt2=kt2)

            def emit_vload(b):
                # staged V (fp32) with a 65th ones-column per chunk:
                # vs[p, kc*65 + j] = V[b, kc*128 + p, j] for j < 64
                nreal = nreals[b]
                vs = residp.tile(
                    [128, KC * (D + 1)], F32, tag=f"vs{b}", name=f"vs_{b}"
                )
                vsrc = v[b]  # [S, D]
                nc.sync.dma_start(
                    out=vs[:, 0 : nreal * (D + 1)]
                    .rearrange("p (kc j) -> p kc j", j=D + 1)[:, :, 0:D],
                    in_=bass.AP(
                        tensor=vsrc.tensor,
                        offset=vsrc.offset,
                        ap=[[D, 128], [128 * D, nreal], [1, D]],
                    ),
                )
                slot_res[b]["vs"] = vs

            def emit_vmprep(b, masks):
                nreal = nreals[b]
                vs = slot_res[b]["vs"]
                vs3 = vs[:, 0 : nreal * (D + 1)].rearrange(
                    "p (kc j) -> p kc j", j=D + 1
                )
                # ones into the mask column of each chunk block
                nc.vector.memset(vs3[:, :, D : D + 1], 1.0)
                # vm = (V | ones) * mask  -> bf16, in one op; the mask col
                # becomes the mask value itself (den row of UT)
                vm = residp.tile(
                    [128, KC * (D + 1)], BF16, tag=f"vm{b}", name=f"vm_{b}"
                )
                mcols = masks.rearrange("p (kc b2) -> p kc b2", b2=BPC)[
                    :, 0:nreal, b : b + 1
                ]
                mask_b = bass.AP(
                    tensor=mcols.tensor,
                    offset=mcols.offset,
                    ap=[list(d) for d in mcols.ap[:-1]] + [[0, D + 1]],
                )
                nc.vector.tensor_tensor(
                    out=vm[:, 0 : nreal * (D + 1)].rearrange(
                        "p (kc j) -> p kc j", j=D + 1
                    ),
                    in0=vs3,
                    in1=mask_b,
                    op=ALU.mult,
                )
                slot_res[b]["vm"] = vm

            # ---- per-(k-chunk, slot) 0/1 masks from valid_lens --------
            # masks[p, kc*BPC + b] = 1.0 if kc*128 + p < vl[b] else 0.0
            # vl goes via the gpsimd DMA queue so it isn't stuck behind the
            # bulk input DMAs on the sync queues
            vl_i = constp.tile([1, BPC], I32)
            nc.gpsimd.dma_start(out=vl_i, in_=vl)

            # qt/kt for the first slots, then their V, then the rest —
            # ordered so data arrives just ahead of its use
            for b in slot_order[0:2]:
                emit_inputs(b)
            for b in slot_order[0:2]:
                emit_vload(b)
            for b in slot_order[2:]:
                emit_inputs(b)
                emit_vload(b)

            vl_f1 = constp.tile([1, BPC], F32)
            nc.vector.tensor_copy(out=vl_f1[:], in_=vl_i[:])
            vl_bf = constp.tile([128, BPC], F32)
            nc.gpsimd.partition_broadcast(vl_bf[:], vl_f1[:])
            iota_i = constp.tile([128, 1], I32)
            nc.gpsimd.iota(iota_i[:], pattern=[[0, 1]], base=0, channel_multiplier=1)
            iota_f = constp.tile([128, 1], F32)
            nc.vector.tensor_copy(out=iota_f[:], in_=iota_i[:])
            u = constp.tile([128, BPC], F32)  # u[p, b] = vl[b] - p
            nc.vector.tensor_scalar(
                out=u[:],
                in0=vl_bf[:],
                scalar1=iota_f[:],
                scalar2=None,
                op0=ALU.subtract,
            )
            masks = constp.tile([128, KC * BPC], F32)
            for kc in range(KC):
                msl = masks[:, kc * BPC : (kc + 1) * BPC]
                nc.vector.tensor_scalar(
                    out=msl,
                    in0=u[:],
                    scalar1=float(kc * 128),
                    scalar2=1.0,
                    op0=ALU.subtract,
                    op1=ALU.min,
                )
                nc.vector.tensor_scalar(
                    out=msl, in0=msl, scalar1=0.0, scalar2=None, op0=ALU.max
                )

            emit_vmprep(slot_order[0], masks)
            emit_vmprep(slot_order[1], masks)

            # ---- steady-state pipeline --------------------------------
            pending = []  # deferred postprocess stages, popped per group

            def pump(n=1):
                for _ in range(n):
                    if pending:
                        pending.pop(0)()

            def make_post(b, h, utq, osb):
                """utq: [128, 4*65] PSUM — 4 q-chunks of half h, [q, d|den]
                layout. den is per-PARTITION (column 64 of each 65-block),
                so recip is a tiny [128,4] op and the normalize multiply is
                ONE tensor_tensor over the whole half with the reciprocal
                broadcast along the free dim. osb [128, 8*65] includes the
                (useless) scaled den columns; the output DMA skips them and
                writes ot[b] in [S, D] layout."""
                den_sb = postp.tile([128, 4], F32, tag="den")
                rec = postp.tile([128, 4], F32, tag="rec")
                utq3 = utq.rearrange("p (jj c) -> p jj c", c=D + 1)

                def d1():
                    # strided den-column gather must be DVE (ACT mis-reads
                    # inner-strided APs)
                    nc.vector.tensor_copy(out=den_sb[:], in_=utq3[:, :, D : D + 1])
                    nc.vector.reciprocal_approx_fast(rec[:], den_sb[:])

                def d2():
                    rsl = rec[:]
                    rec_b = bass.AP(
                        tensor=rsl.tensor,
                        offset=rsl.offset,
                        ap=[list(dm) for dm in rsl.ap] + [[0, D + 1]],
                    )
                    nc.vector.tensor_tensor(
                        out=osb[:, h * 4 * (D + 1) : (h + 1) * 4 * (D + 1)]
                        .rearrange("p (jj c) -> p jj c", c=D + 1),
                        in0=utq3,
                        in1=rec_b,
                        op=ALU.mult,
                    )

                def d3():
                    if h == 1:
                        nc.gpsimd.dma_start(out=ot[b], in_=osb[:])

                return [d1, d2, d3] if h == 1 else [d1, d2]

            # flat group list: (slot_idx, b, h, p, first_of_slot_group_no)
            groups = []
            for si, b in enumerate(slot_order):
                npairs = nprocs[b] // 2
                for h in range(2):
                    for p in range(npairs):
                        groups.append((si, b, h, p))

            pendq = []  # [(emit_utt_fn, post_stages_or_None)]
            gi_of_slot = {}
            for gi, (si, b, h, p) in enumerate(groups):
                if si not in gi_of_slot:
                    gi_of_slot[si] = gi
                npairs = nprocs[b] // 2
                nreal = nreals[b]
                qt2 = slot_res[b]["qt2"]
                kt2 = slot_res[b]["kt2"]
                hs = slice(h * QH, (h + 1) * QH)

                if h == 0 and p == 0:
                    ut_h = {}
                    osb = postp.tile([128, KC * (D + 1)], F32, tag="osb")
                utq = ut_h.get(h)
                if utq is None:
                    utq = utp.tile([128, 4 * (D + 1)], F32, tag="utq")
                    ut_h[h] = utq

                # vm prep for the next slot, deferred into the pending queue
                # so its wait on the V DMA can't head-of-line-block the DVE
                if gi == gi_of_slot[si] + 1 and si + 2 < len(slot_order):
                    pending.append(
                        lambda b2=slot_order[si + 2]: emit_vmprep(b2, masks)
                    )

                st = stp.tile([128, 2 * QH], F32, tag="st")
                nc.tensor.matmul(
                    st[:, 0:QH],
                    kt2[0:64, 2 * p * 128 : (2 * p + 1) * 128],
                    qt2[0:64, hs],
                    start=True,
                    stop=True,
                    tile_position=(0, 0),
                )
                if 2 * p + 1 < nreal:
                    nc.tensor.matmul(
                        st[:, QH : 2 * QH],
                        kt2[64:128, (2 * p + 1) * 128 : (2 * p + 2) * 128],
                        qt2[64:128, hs],
                        start=True,
                        stop=True,
                        tile_position=(64, 0),
                    )
                    wid = 2 * QH
                else:
                    wid = QH

                if DVE_EXP and p == npairs - 1 and npairs >= MIN_PAIRS_FOR_DVE:
                    esti = estp.tile([128, 2 * QH], I16, tag="esti")
                    nc.vector.tensor_scalar(
                        out=esti[:, 0:wid],
                        in0=st[:, 0:wid],
                        scalar1=SCH_A,
                        scalar2=SCH_B,
                        op0=ALU.mult,
                        op1=ALU.add,
                    )
                    est_ap = esti.bitcast(BF16)
                else:
                    est = estp.tile([128, 2 * QH], BF16, tag="est")
                    nc.scalar.activation(
                        out=est[:, 0:wid], in_=st[:, 0:wid], func=AF.Exp, scale=0.125
                    )
                    est_ap = est

                vm = slot_res[b]["vm"]

                def emit_utt(utq=utq, est_ap=est_ap, p=p, nreal=nreal, vm=vm):
                    # UTT[q, d|den] += est_chunk.T @ vm_chunk, per 128-q
                    # sub-chunk jj of this half (est block is the 128x128
                    # stationary operand; vm [128, 65] moves).
                    # start=True clears has_written for the WHOLE bank, so
                    # only the very first MM into this utq tile may set it;
                    # later regions' first writes overwrite via cleared bits.
                    for jj in range(4):
                        for kcl in range(2):
                            kc = 2 * p + kcl
                            if kc >= nreal:
                                continue
                            nc.tensor.matmul(
                                utq[:, jj * (D + 1) : (jj + 1) * (D + 1)],
                                est_ap[:, kcl * QH + jj * 128 : kcl * QH + (jj + 1) * 128],
                                vm[:, kc * (D + 1) : (kc + 1) * (D + 1)],
                                start=(kc == 0 and jj == 0),
                                stop=(kc == nreal - 1),
                                skip_group_check=True,
                            )

                # UTT deferred one group so the next group's STs sit ahead
                # of each est-wait in the PE's in-order queue
                is_last_of_half = p == npairs - 1
                pendq.append(
                    (emit_utt, make_post(b, h, utq, osb) if is_last_of_half else None)
                )
                if len(pendq) > 1:
                    eu, post = pendq.pop(0)
                    eu()
                    if post is not None:
                        pending.extend(post)
                pump(2)

            while pendq:
                eu, post = pendq.pop(0)
                eu()
                if post is not None:
                    pending.extend(post)
            while pending:
                pump(1)

    nc.compile()
    return nc


def _plan(valid_lens):
    """Sort batches by length, deal to (slot, core); per-slot chunk counts."""
    order = np.argsort(-valid_lens, kind="stable")  # [B]
    nprocs, nreals = [], []
    for s in range(BPC):
        slot_max = int(valid_lens[order[s * NCORES]])
        nchunks = max(1, -(-slot_max // 128))  # ceil, >= 1
        npc = max(2, min(KC, 2 * ((nchunks + 1) // 2)))
        nprocs.append(npc)
        nreals.append(min(nchunks, npc))
    return order, tuple(nprocs), tuple(nreals)


def kernel(query, key, value, valid_lens):
    query = np.ascontiguousarray(np.asarray(query, dtype=np.float32))
    key = np.ascontiguousarray(np.asarray(key, dtype=np.float32))
    value = np.ascontiguousarray(np.asarray(value, dtype=np.float32))
    valid_lens = np.asarray(valid_lens).astype(np.int32).reshape(B)
    assert query.shape == (B, S, D) and key.shape == (B, S, D)
    assert value.shape == (B, S, D)

    order, nprocs, nreals = _plan(valid_lens)
    cache_key = (nprocs, nreals, DVE_EXP, MIN_PAIRS_FOR_DVE)
    nc = _BUILD_CACHE.get(cache_key)
    if nc is None:
        nc = _build(nprocs, nreals)
        _BUILD_CACHE[cache_key] = nc

    qt = query.transpose(0, 2, 1)  # views
    kt = key.transpose(0, 2, 1)
    in_maps = []
    for c in range(NCORES):
        idx = [int(order[s * NCORES + c]) for s in range(BPC)]
        in_maps.append(
            {
                "qt": np.ascontiguousarray(qt[idx]),
                "kt": np.ascontiguousarray(kt[idx]),
                "v": np.ascontiguousarray(value[idx]),
                "vl": np.ascontiguousarray(valid_lens[idx].reshape(1, BPC)),
            }
        )

    res = bass_utils.run_bass_kernel_spmd(
        nc, in_maps, core_ids=list(range(NCORES)), **_RUN_KWARGS
    )
    global _LAST_RES
    _LAST_RES = res

    out = np.empty((B, S, D), dtype=np.float32)
    for c in range(NCORES):
        otc = res.results[c]["ot"]  # [BPC, 128, KC*(D+1)] blocked
        for s in range(BPC):
            blk = otc[s].reshape(128, KC, D + 1)[:, :, 0:D]  # [p, jj, d]
            out[int(order[s * NCORES + c])] = blk.transpose(1, 0, 2).reshape(S, D)
    return out
